# revision 26
# baseline (speedup 1.0000x reference)
"""Trainium2 8-core kernel for the paired contrastive (NT-Xent-like) loss.

Math (tau=0.5, N=8192, D=256):
    z1 = l2norm(H_1), z2 = l2norm(H_2)
    den1_i = sum_j exp(z1.z1/t) + sum_j exp(z1.z2/t) - exp(s11_ii/t)
    den2_i = sum_j exp(z2.z2/t) + sum_j exp(z2.z1/t) - exp(s22_ii/t)
    loss = (1/2N) * sum_i [ ln(den1_i) + ln(den2_i) - 2*(z1_i.z2_i)/t ]

S11 and S22 are symmetric, so only their upper triangles are computed
(2N^2 exps instead of 3N^2).  Work is balanced by pairing 128-row tiles:
row-tile r in [0,32) computes cyclic column distances 0..32, r in [32,64)
distances 0..31 -- every unordered tile pair is covered exactly once.

Each core owns 8 row-tiles {4c+u} u {32+4c+u} (u<4).  The host hands every
core the l2-normalized embeddings quantized to fp8-e4m3 (x64), columns
ROTATED by 4c tiles plus 3 duplicated tiles appended, so each core's
stationary blocks and triangle windows sit at the same local offsets and
the SPMD graph is identical across cores.

Device dataflow per 2048-column group:
  TensorE: fp8 DoubleRow matmul (full D=256 contraction per pass) -> PSUM
  ScalarE: exp via ACT (scale folds the fp8 scaling and 1/tau), bf16 out,
           fused fp32 row-sum accumulator (accum_out)
  DVE:     bf16 add of the exp tile into the per-core column accumulator
           (skipped for the first slot touching a region: ACT writes the
           column accumulator directly)
The four ragged 128-col window tails of each symmetric stream are batched
into ONE 512-wide group (4 small matmuls into one PSUM bank; the tails are
column-contiguous so one DVE add covers them and a strided DVE reduce
recovers the 4 per-slot row sums) -- this keeps the ScalarE stream free of
short instructions and the stall they caused at slot boundaries.

The column accumulators and row-sum tiles are DMA'd out (progressively,
so the post-stream tail is tiny); the host does the cross-partition /
cross-core reductions in f64, the exact diagonal corrections, the exact
z1.z2 diagonal term, and the final log/mean.  ScalarE runs nothing but
the 66 exp ACTIVATEs -- it is the critical path (~16.5M exps/core at
1 elem/lane/cycle).
"""

import numpy as np
import ml_dtypes

import concourse.bass as bass
import concourse.tile as tile
from concourse import bacc, mybir
from concourse.bass_utils import run_bass_kernel_spmd

F32 = mybir.dt.float32
BF16 = mybir.dt.bfloat16
F8 = mybir.dt.float8e4
AF = mybir.ActivationFunctionType
ALU = mybir.AluOpType
AX = mybir.AxisListType
DR = mybir.MatmulPerfMode.DoubleRow

TAU = 0.5
SCALE = 64.0                 # fp8 quantization scale for z
ACT_SCALE = 1.0 / (TAU * SCALE * SCALE)   # exp(ACT_SCALE * q_i.q_j)

N_FULL, D_FULL, N_CORES = 8192, 256, 8
TI = 128
T = N_FULL // TI             # 64 column tiles
EXTT = T + 3                 # 3 duplicated tiles so every window is contiguous
EXTC = EXTT * TI             # 8576
G = 2048                     # exp group (4 PSUM banks)
CH = 512                     # matmul chunk (one PSUM bank)

# stationary slots: local tile positions 0..3 and 32..35
SPOS = [TI * u for u in range(4)] + [4096 + TI * u for u in range(4)]

# slot iteration orders; the first slots listed write their exp output
# straight into the column accumulator (fresh region, no DVE add)
ORDER = {"s12": [0, 1, 2, 3, 4, 5, 6, 7], "sym": [4, 0, 5, 6, 7, 1, 2, 3]}
DIRECT = {"s12": {0}, "sym": {4, 0}}

RAG0, RAGW = 4096, 512       # merged ragged tails cover cols [4096, 4608)


def _windows(st, s):
    """(start, width) of the full-group window for stream st, slot s."""
    if st == "s12":
        return 0, N_FULL
    if s < 4:
        return TI * s, 2 * G         # ragged 33rd tile handled separately
    return 4096 + TI * (s - 4), 2 * G


def _acc_col(st, s, gi):
    """Column of the row-sum accumulator tile for (stream, slot, group)."""
    if st == "s12":
        return 4 * s + gi
    base = 32 if st == "s22" else 48
    return base + 2 * s + gi


def _rag_col(st, u):
    return (64 if st == "s22" else 68) + u


def build_nc(n_cores=N_CORES):
    nc = bacc.Bacc("TRN2", target_bir_lowering=False, debug=False,
                   num_devices=n_cores)

    m_in = [nc.dram_tensor("M1", [TI, 2, EXTC], F8, kind="ExternalInput"),
            nc.dram_tensor("M2", [TI, 2, EXTC], F8, kind="ExternalInput")]
    ca_out = {"s12": nc.dram_tensor("ca12", [TI, N_FULL], BF16,
                                    kind="ExternalOutput"),
              "s22": nc.dram_tensor("ca22", [TI, EXTC], BF16,
                                    kind="ExternalOutput"),
              "s11": nc.dram_tensor("ca11", [TI, EXTC], BF16,
                                    kind="ExternalOutput")}
    acc_out = nc.dram_tensor("accs", [TI, 72], F32, kind="ExternalOutput")

    with tile.TileContext(nc) as tc, \
         tc.tile_pool(name="persist", bufs=1) as per:
        Q = [per.tile([TI, 2, EXTC], F8, tag=f"q{t}", name=f"q{t}")
             for t in range(2)]
        CA = {"s12": per.tile([TI, N_FULL], BF16, tag="ca12", name="ca12"),
              "s22": per.tile([TI, EXTC], BF16, tag="ca22", name="ca22"),
              "s11": per.tile([TI, EXTC], BF16, tag="ca11", name="ca11")}
        acc = per.tile([TI, 72], F32, tag="acc", name="acc")

        # ---- input DMA in consumption order on the two HWDGE queues
        # (k0 on sync, k1 on scalar -- scalar is idle before its first
        # ACTIVATE).  The s12 sweep is column-major, so its first ~17us
        # of ACT work only consumes the Q1 stationaries [0:512) plus Q2
        # cols [0:2048) -- that prefix leads both queues; later passes'
        # columns stream in far ahead of consumption, with the s11-only
        # M1 bulk last.  The gpsimd SWDGE moves nothing in this kernel,
        # which keeps the slow SWDGE drain out of the epilogue.
        # scalar rings at most 3 doorbells: more would exceed the HWDGE
        # queue depth and block the engine (delaying its first ACTIVATE)
        # while it waits to ring the rest; sync (otherwise idle) takes
        # everything else in consumption order.
        nc.sync.dma_start(Q[0][:, 0, 0:512], m_in[0].ap()[:, 0, 0:512])
        nc.scalar.dma_start(Q[0][:, 1, 0:512], m_in[0].ap()[:, 1, 0:512])
        nc.sync.dma_start(Q[1][:, 0, 0:2048], m_in[1].ap()[:, 0, 0:2048])
        nc.scalar.dma_start(Q[1][:, 1, 0:2048], m_in[1].ap()[:, 1, 0:2048])
        nc.scalar.dma_start(Q[1][:, 1, 2048:4096],
                            m_in[1].ap()[:, 1, 2048:4096])
        rest = [(1, 0, 2048, 4096),
                (1, 0, 4096, 6144), (1, 1, 4096, 6144),
                (1, 0, 6144, EXTC), (1, 1, 6144, EXTC),
                (0, 0, 512, 2560), (0, 1, 512, 2560),
                (0, 0, 2560, 4608), (0, 1, 2560, 4608),
                (0, 0, 4608, EXTC), (0, 1, 4608, EXTC)]
        for t, k, c0, c1 in rest:
            nc.sync.dma_start(Q[t][:, k, c0:c1], m_in[t].ap()[:, k, c0:c1])
        # the 3 duplicated tiles of the symmetric accumulators are touched
        # only by late slots' window tails -- zero them up front
        nc.gpsimd.memset(CA["s22"][:, N_FULL:EXTC], 0.0)
        nc.gpsimd.memset(CA["s11"][:, N_FULL:EXTC], 0.0)

        with tc.tile_pool(name="spool", bufs=2, space="PSUM") as spool, \
             tc.tile_pool(name="escp", bufs=6) as escp:

            # ---- PE warm-up: the first real matmul chain otherwise runs
            # on a cold PE (low p-state) right when the first ACTIVATE is
            # already data-gated.  The memset region of ca22 is available
            # ~10us before the DMA prefix lands, so a run of dummy fp8
            # matmuls over those zeros keeps the PE busy (and ramping)
            # through the DMA wait.  Output is never read.
            wsrc = CA["s22"][:, N_FULL:EXTC].bitcast(F8)
            wl = wsrc[:, 0:256].rearrange("p (k m) -> p k m", k=2)
            wr = wsrc[:, 0:768].rearrange("p (k m) -> p k m", k=2)
            warm = spool.tile([TI, G], F32, tag="sg", name="sg")
            for _ in range(24):
                nc.tensor.matmul(warm[:, 0:384], wl, wr,
                                 start=True, stop=True, perf_mode=DR)

            def exp_group(st, QS, QM, s, g0, gw, ai, direct):
                so = SPOS[s]
                ca = CA[st]
                sg = spool.tile([TI, G], F32, tag="sg", name="sg")
                for o in range(0, gw, CH):
                    nc.tensor.matmul(sg[:, o:o + CH],
                                     Q[QS][:, :, so:so + TI],
                                     Q[QM][:, :, g0 + o:g0 + o + CH],
                                     start=True, stop=True, perf_mode=DR)
                if direct:
                    nc.scalar.activation(ca[:, g0:g0 + gw], sg[:, :gw],
                                         AF.Exp, bias=0.0, scale=ACT_SCALE,
                                         accum_out=acc[:, ai:ai + 1])
                else:
                    esc = escp.tile([TI, G], BF16, tag="esc", name="esc")
                    nc.scalar.activation(esc[:, :gw], sg[:, :gw],
                                         AF.Exp, bias=0.0, scale=ACT_SCALE,
                                         accum_out=acc[:, ai:ai + 1])
                    nc.vector.tensor_add(ca[:, g0:g0 + gw],
                                         ca[:, g0:g0 + gw], esc[:, :gw])

            def rag_group(st, QQ):
                # the four 33rd-tile tails (slots u<4) merged into one
                # 512-wide group; their columns tile [4096, 4608) exactly
                ca = CA[st]
                sg = spool.tile([TI, G], F32, tag="sg", name="sg")
                for u in range(4):
                    mv = RAG0 + TI * u
                    nc.tensor.matmul(sg[:, TI * u:TI * (u + 1)],
                                     Q[QQ][:, :, SPOS[u]:SPOS[u] + TI],
                                     Q[QQ][:, :, mv:mv + TI],
                                     start=True, stop=True, perf_mode=DR)
                esc = escp.tile([TI, G], BF16, tag="esc", name="esc")
                nc.scalar.activation(esc[:, :RAGW], sg[:, :RAGW],
                                     AF.Exp, bias=0.0, scale=ACT_SCALE)
                nc.vector.tensor_add(ca[:, RAG0:RAG0 + RAGW],
                                     ca[:, RAG0:RAG0 + RAGW], esc[:, :RAGW])
                r0 = _rag_col(st, 0)
                nc.vector.tensor_reduce(
                    acc[:, r0:r0 + 4],
                    esc[:, :RAGW].rearrange("p (u w) -> p u w", u=4),
                    AX.X, ALU.add)

            def run_slot(st, QS, QM, s):
                w0, ww = _windows(st, s)
                direct = s in DIRECT["sym"]
                for gi in range(ww // G):
                    exp_group(st, QS, QM, s, w0 + G * gi, G,
                              _acc_col(st, s, gi), direct)

            # ---------------- s12 (column-major: all slots share each
            # 2048-col window, so pass 0 only needs the DMA prefix) ------
            for gi in range(4):
                for s in ORDER["s12"]:
                    exp_group("s12", 0, 1, s, G * gi, G,
                              _acc_col("s12", s, gi), s in DIRECT["s12"])
            nc.sync.dma_start(ca_out["s12"].ap()[:, 0:4096],
                              CA["s12"][:, 0:4096])
            nc.sync.dma_start(ca_out["s12"].ap()[:, 4096:N_FULL],
                              CA["s12"][:, 4096:N_FULL])

            # ---------------- s22 ----------------
            for i, s in enumerate(ORDER["sym"]):
                run_slot("s22", 1, 1, s)
                if i == 4:   # cols >= 4608 final after slots 4,0,5,6,7
                    nc.sync.dma_start(ca_out["s22"].ap()[:, 4608:EXTC],
                                      CA["s22"][:, 4608:EXTC])
            nc.sync.dma_start(ca_out["s22"].ap()[:, 0:2304],
                              CA["s22"][:, 0:2304])
            nc.sync.dma_start(ca_out["s22"].ap()[:, 2304:4096],
                              CA["s22"][:, 2304:4096])

            # ---------------- s11 (tail-critical: progressive DMA) ------
            for i, s in enumerate(ORDER["sym"][:-1]):
                run_slot("s11", 0, 0, s)
                if i == 4:
                    nc.sync.dma_start(ca_out["s11"].ap()[:, 4608:EXTC],
                                      CA["s11"][:, 4608:EXTC])
            # last slot (3): DMA each region right after it finalizes
            exp_group("s11", 0, 0, 3, 384, G, _acc_col("s11", 3, 0),
                      False)                         # [384, 2432)
            nc.sync.dma_start(ca_out["s11"].ap()[:, 0:1216],
                              CA["s11"][:, 0:1216])
            nc.sync.dma_start(ca_out["s11"].ap()[:, 1216:2432],
                              CA["s11"][:, 1216:2432])
            exp_group("s11", 0, 0, 3, 384 + G, G, _acc_col("s11", 3, 1),
                      False)                         # [2432, 4480)
            nc.sync.dma_start(ca_out["s11"].ap()[:, 2432:4096],
                              CA["s11"][:, 2432:4096])
            rag_group("s22", 1)                    # [4096, 4608)
            nc.sync.dma_start(ca_out["s22"].ap()[:, 4096:4608],
                              CA["s22"][:, 4096:4608])
            rag_group("s11", 0)                    # [4096, 4608)
            nc.scalar.dma_start(ca_out["s11"].ap()[:, 4096:4608],
                                CA["s11"][:, 4096:4608])
            nc.scalar.dma_start(acc_out.ap()[:, :], acc[:])

    nc.compile()
    return nc


_CACHE = {}


def _compiled(n_cores=N_CORES):
    if n_cores not in _CACHE:
        _CACHE[n_cores] = build_nc(n_cores)
    return _CACHE[n_cores]


def _perm(c):
    p = np.arange(EXTC)
    return TI * ((4 * c + p // TI) % T) + p % TI


def _quantize(H):
    H = np.asarray(H, np.float32)
    z = H / np.maximum(np.sqrt((H * H).sum(1, keepdims=True)), 1e-12)
    q = (z * SCALE).astype(ml_dtypes.float8_e4m3)
    return z, q


def make_in_maps(H_1, H_2, n_cores=N_CORES):
    _, q1 = _quantize(H_1)
    _, q2 = _quantize(H_2)
    # [N, D] -> [D, N] -> [2, TI, N] -> [TI, 2, N]
    qt1 = q1.T.reshape(2, TI, N_FULL).transpose(1, 0, 2)
    qt2 = q2.T.reshape(2, TI, N_FULL).transpose(1, 0, 2)
    maps = []
    for c in range(n_cores):
        pm = _perm(c)
        maps.append({"M1": np.ascontiguousarray(qt1[:, :, pm]),
                     "M2": np.ascontiguousarray(qt2[:, :, pm])})
    return maps


def finalize(results, H_1, H_2, n_cores=N_CORES):
    N = N_FULL
    z1, q1 = _quantize(H_1)
    z2, q2 = _quantize(H_2)
    den1 = np.zeros(N, np.float64)
    den2 = np.zeros(N, np.float64)
    for c in range(n_cores):
        r = results[c]
        pm = _perm(c)
        A = np.asarray(r["accs"], np.float64)
        for s in range(8):
            rt = 4 * c + s if s < 4 else 32 + 4 * c + (s - 4)
            gr = slice(TI * rt, TI * (rt + 1))
            den1[gr] += A[:, [_acc_col("s12", s, g)
                              for g in range(4)]].sum(1)
            den1[gr] += A[:, [_acc_col("s11", s, g) for g in range(2)]].sum(1)
            den2[gr] += A[:, [_acc_col("s22", s, g) for g in range(2)]].sum(1)
            if s < 4:
                den1[gr] += A[:, _rag_col("s11", s)]
                den2[gr] += A[:, _rag_col("s22", s)]
        np.add.at(den2, pm[:N],
                  np.asarray(r["ca12"], np.float64).sum(0))
        np.add.at(den1, pm, np.asarray(r["ca11"], np.float64).sum(0))
        np.add.at(den2, pm, np.asarray(r["ca22"], np.float64).sum(0))
    # exact diagonal corrections for the quantized Gram diagonals
    qf1 = q1.astype(np.float64)
    qf2 = q2.astype(np.float64)
    den1 -= np.exp(ACT_SCALE * (qf1 * qf1).sum(1))
    den2 -= np.exp(ACT_SCALE * (qf2 * qf2).sum(1))
    ii = float((z1.astype(np.float64) * z2.astype(np.float64)).sum())
    loss = (np.sum(np.log(den1)) + np.sum(np.log(den2))
            - (2.0 / TAU) * ii) / (2.0 * N)
    return np.float32(loss)


def kernel(H_1, H_2):
    nc = _compiled(N_CORES)
    in_maps = make_in_maps(H_1, H_2, N_CORES)
    res = run_bass_kernel_spmd(nc, in_maps, core_ids=list(range(N_CORES)))
    return finalize(res.results, H_1, H_2, N_CORES)


# revision 27
# speedup vs baseline: 1.0102x; 1.0102x over previous
"""Trainium2 8-core kernel for the paired contrastive (NT-Xent-like) loss.

Math (tau=0.5, N=8192, D=256):
    z1 = l2norm(H_1), z2 = l2norm(H_2)
    den1_i = sum_j exp(z1.z1/t) + sum_j exp(z1.z2/t) - exp(s11_ii/t)
    den2_i = sum_j exp(z2.z2/t) + sum_j exp(z2.z1/t) - exp(s22_ii/t)
    loss = (1/2N) * sum_i [ ln(den1_i) + ln(den2_i) - 2*(z1_i.z2_i)/t ]

S11 and S22 are symmetric, so only their upper triangles are computed
(2N^2 exps instead of 3N^2).  Work is balanced by pairing 128-row tiles:
row-tile r in [0,32) computes cyclic column distances 0..32, r in [32,64)
distances 0..31 -- every unordered tile pair is covered exactly once.

Each core owns 8 row-tiles {4c+u} u {32+4c+u} (u<4).  The host hands every
core the l2-normalized embeddings quantized to fp8-e4m3 (x64), columns
ROTATED by 4c tiles plus 3 duplicated tiles appended, so each core's
stationary blocks and triangle windows sit at the same local offsets and
the SPMD graph is identical across cores.

Device dataflow per 2048-column group:
  TensorE: fp8 DoubleRow matmul (full D=256 contraction per pass) -> PSUM
  ScalarE: exp via ACT (scale folds the fp8 scaling and 1/tau), bf16 out,
           fused fp32 row-sum accumulator (accum_out)
  DVE:     bf16 add of the exp tile into the per-core column accumulator
           (skipped for the first slot touching a region: ACT writes the
           column accumulator directly)
The four ragged 128-col window tails of each symmetric stream are batched
into ONE 512-wide group (4 small matmuls into one PSUM bank; the tails are
column-contiguous so one DVE add covers them and a strided DVE reduce
recovers the 4 per-slot row sums) -- this keeps the ScalarE stream free of
short instructions and the stall they caused at slot boundaries.

The column accumulators and row-sum tiles are DMA'd out (progressively,
so the post-stream tail is tiny); the host does the cross-partition /
cross-core reductions in f64, the exact diagonal corrections, the exact
z1.z2 diagonal term, and the final log/mean.  ScalarE runs nothing but
the 66 exp ACTIVATEs -- it is the critical path (~16.5M exps/core at
1 elem/lane/cycle).
"""

import numpy as np
import ml_dtypes

import concourse.bass as bass
import concourse.tile as tile
from concourse import bacc, mybir
from concourse.bass_utils import run_bass_kernel_spmd

F32 = mybir.dt.float32
BF16 = mybir.dt.bfloat16
F8 = mybir.dt.float8e4
AF = mybir.ActivationFunctionType
ALU = mybir.AluOpType
AX = mybir.AxisListType
DR = mybir.MatmulPerfMode.DoubleRow

TAU = 0.5
SCALE = 64.0                 # fp8 quantization scale for z
ACT_SCALE = 1.0 / (TAU * SCALE * SCALE)   # exp(ACT_SCALE * q_i.q_j)

N_FULL, D_FULL, N_CORES = 8192, 256, 8
TI = 128
T = N_FULL // TI             # 64 column tiles
EXTT = T + 3                 # 3 duplicated tiles so every window is contiguous
EXTC = EXTT * TI             # 8576
G = 2048                     # exp group (4 PSUM banks)
CH = 512                     # matmul chunk (one PSUM bank)

# stationary slots: local tile positions 0..3 and 32..35
SPOS = [TI * u for u in range(4)] + [4096 + TI * u for u in range(4)]

# slot iteration orders; the first slots listed write their exp output
# straight into the column accumulator (fresh region, no DVE add)
ORDER = {"s12": [0, 1, 2, 3, 4, 5, 6, 7], "sym": [4, 0, 5, 6, 7, 1, 2, 3]}
DIRECT = {"s12": {0}, "sym": {4, 0}}

RAG0, RAGW = 4096, 512       # merged ragged tails cover cols [4096, 4608)


def _windows(st, s):
    """(start, width) of the full-group window for stream st, slot s."""
    if st == "s12":
        return 0, N_FULL
    if s < 4:
        return TI * s, 2 * G         # ragged 33rd tile handled separately
    return 4096 + TI * (s - 4), 2 * G


def _acc_col(st, s, gi):
    """Column of the row-sum accumulator tile for (stream, slot, group)."""
    if st == "s12":
        return 4 * s + gi
    base = 32 if st == "s22" else 48
    return base + 2 * s + gi


def _rag_col(st, u):
    return (64 if st == "s22" else 68) + u


def build_nc(n_cores=N_CORES):
    nc = bacc.Bacc("TRN2", target_bir_lowering=False, debug=False,
                   num_devices=n_cores)

    m_in = [nc.dram_tensor("M1", [TI, 2, EXTC], F8, kind="ExternalInput"),
            nc.dram_tensor("M2", [TI, 2, EXTC], F8, kind="ExternalInput")]
    ca_out = {"s12": nc.dram_tensor("ca12", [TI, N_FULL], BF16,
                                    kind="ExternalOutput"),
              "s22": nc.dram_tensor("ca22", [TI, EXTC], BF16,
                                    kind="ExternalOutput"),
              "s11": nc.dram_tensor("ca11", [TI, EXTC], BF16,
                                    kind="ExternalOutput")}
    acc_out = nc.dram_tensor("accs", [TI, 72], F32, kind="ExternalOutput")

    with tile.TileContext(nc) as tc, \
         tc.tile_pool(name="persist", bufs=1) as per:
        Q = [per.tile([TI, 2, EXTC], F8, tag=f"q{t}", name=f"q{t}")
             for t in range(2)]
        CA = {"s12": per.tile([TI, N_FULL], BF16, tag="ca12", name="ca12"),
              "s22": per.tile([TI, EXTC], BF16, tag="ca22", name="ca22"),
              "s11": per.tile([TI, EXTC], BF16, tag="ca11", name="ca11")}
        acc = per.tile([TI, 72], F32, tag="acc", name="acc")

        # ---- input DMA in consumption order on the two HWDGE queues
        # (k0 on sync, k1 on scalar -- scalar is idle before its first
        # ACTIVATE).  The s12 sweep is column-major, so its first ~17us
        # of ACT work only consumes the Q1 stationaries [0:512) plus Q2
        # cols [0:2048) -- that prefix leads both queues; later passes'
        # columns stream in far ahead of consumption, with the s11-only
        # M1 bulk last.  The gpsimd SWDGE moves nothing in this kernel,
        # which keeps the slow SWDGE drain out of the epilogue.
        # scalar rings at most 3 doorbells: more would exceed the HWDGE
        # queue depth and block the engine (delaying its first ACTIVATE)
        # while it waits to ring the rest; sync (otherwise idle) takes
        # everything else in consumption order.
        nc.sync.dma_start(Q[0][:, 0, 0:512], m_in[0].ap()[:, 0, 0:512])
        nc.scalar.dma_start(Q[0][:, 1, 0:512], m_in[0].ap()[:, 1, 0:512])
        # two half-transfers per queue: the group-0 matmul chunks depend
        # on per-transfer ranges, so chunks 0-1 start as soon as the first
        # half lands, overlapping the second half's transfer
        nc.sync.dma_start(Q[1][:, 0, 0:1024], m_in[1].ap()[:, 0, 0:1024])
        nc.scalar.dma_start(Q[1][:, 1, 0:1024], m_in[1].ap()[:, 1, 0:1024])
        nc.sync.dma_start(Q[1][:, 0, 1024:2048],
                          m_in[1].ap()[:, 0, 1024:2048])
        nc.scalar.dma_start(Q[1][:, 1, 1024:2048],
                            m_in[1].ap()[:, 1, 1024:2048])
        nc.scalar.dma_start(Q[1][:, 1, 2048:4096],
                            m_in[1].ap()[:, 1, 2048:4096])
        rest = [(1, 0, 2048, 4096),
                (1, 0, 4096, 6144), (1, 1, 4096, 6144),
                (1, 0, 6144, EXTC), (1, 1, 6144, EXTC),
                (0, 0, 512, 2560), (0, 1, 512, 2560),
                (0, 0, 2560, 4608), (0, 1, 2560, 4608),
                (0, 0, 4608, EXTC), (0, 1, 4608, EXTC)]
        for t, k, c0, c1 in rest:
            nc.sync.dma_start(Q[t][:, k, c0:c1], m_in[t].ap()[:, k, c0:c1])
        # the 3 duplicated tiles of the symmetric accumulators are touched
        # only by late slots' window tails -- zero them up front
        nc.gpsimd.memset(CA["s22"][:, N_FULL:EXTC], 0.0)
        nc.gpsimd.memset(CA["s11"][:, N_FULL:EXTC], 0.0)

        with tc.tile_pool(name="spool", bufs=2, space="PSUM") as spool, \
             tc.tile_pool(name="escp", bufs=6) as escp:

            # ---- PE warm-up: the first real matmul chain otherwise runs
            # on a cold PE (low p-state) right when the first ACTIVATE is
            # already data-gated.  The memset region of ca22 is available
            # ~10us before the DMA prefix lands, so a run of dummy fp8
            # matmuls over those zeros keeps the PE busy (and ramping)
            # through the DMA wait.  Output is never read.
            wsrc = CA["s22"][:, N_FULL:EXTC].bitcast(F8)
            wl = wsrc[:, 0:256].rearrange("p (k m) -> p k m", k=2)
            wr = wsrc[:, 0:768].rearrange("p (k m) -> p k m", k=2)
            warm = spool.tile([TI, G], F32, tag="sg", name="sg")
            for _ in range(24):
                nc.tensor.matmul(warm[:, 0:384], wl, wr,
                                 start=True, stop=True, perf_mode=DR)

            def exp_group(st, QS, QM, s, g0, gw, ai, direct):
                so = SPOS[s]
                ca = CA[st]
                sg = spool.tile([TI, G], F32, tag="sg", name="sg")
                for o in range(0, gw, CH):
                    nc.tensor.matmul(sg[:, o:o + CH],
                                     Q[QS][:, :, so:so + TI],
                                     Q[QM][:, :, g0 + o:g0 + o + CH],
                                     start=True, stop=True, perf_mode=DR)
                if direct:
                    nc.scalar.activation(ca[:, g0:g0 + gw], sg[:, :gw],
                                         AF.Exp, bias=0.0, scale=ACT_SCALE,
                                         accum_out=acc[:, ai:ai + 1])
                else:
                    esc = escp.tile([TI, G], BF16, tag="esc", name="esc")
                    nc.scalar.activation(esc[:, :gw], sg[:, :gw],
                                         AF.Exp, bias=0.0, scale=ACT_SCALE,
                                         accum_out=acc[:, ai:ai + 1])
                    nc.vector.tensor_add(ca[:, g0:g0 + gw],
                                         ca[:, g0:g0 + gw], esc[:, :gw])

            def rag_group(st, QQ):
                # the four 33rd-tile tails (slots u<4) merged into one
                # 512-wide group; their columns tile [4096, 4608) exactly
                ca = CA[st]
                sg = spool.tile([TI, G], F32, tag="sg", name="sg")
                for u in range(4):
                    mv = RAG0 + TI * u
                    nc.tensor.matmul(sg[:, TI * u:TI * (u + 1)],
                                     Q[QQ][:, :, SPOS[u]:SPOS[u] + TI],
                                     Q[QQ][:, :, mv:mv + TI],
                                     start=True, stop=True, perf_mode=DR)
                esc = escp.tile([TI, G], BF16, tag="esc", name="esc")
                nc.scalar.activation(esc[:, :RAGW], sg[:, :RAGW],
                                     AF.Exp, bias=0.0, scale=ACT_SCALE)
                nc.vector.tensor_add(ca[:, RAG0:RAG0 + RAGW],
                                     ca[:, RAG0:RAG0 + RAGW], esc[:, :RAGW])
                r0 = _rag_col(st, 0)
                nc.vector.tensor_reduce(
                    acc[:, r0:r0 + 4],
                    esc[:, :RAGW].rearrange("p (u w) -> p u w", u=4),
                    AX.X, ALU.add)

            def run_slot(st, QS, QM, s):
                w0, ww = _windows(st, s)
                direct = s in DIRECT["sym"]
                for gi in range(ww // G):
                    exp_group(st, QS, QM, s, w0 + G * gi, G,
                              _acc_col(st, s, gi), direct)

            # ---------------- s12 (column-major: all slots share each
            # 2048-col window, so pass 0 only needs the DMA prefix) ------
            for gi in range(4):
                for s in ORDER["s12"]:
                    exp_group("s12", 0, 1, s, G * gi, G,
                              _acc_col("s12", s, gi), s in DIRECT["s12"])
            nc.sync.dma_start(ca_out["s12"].ap()[:, 0:4096],
                              CA["s12"][:, 0:4096])
            nc.sync.dma_start(ca_out["s12"].ap()[:, 4096:N_FULL],
                              CA["s12"][:, 4096:N_FULL])

            # ---------------- s22 ----------------
            for i, s in enumerate(ORDER["sym"]):
                run_slot("s22", 1, 1, s)
                if i == 4:   # cols >= 4608 final after slots 4,0,5,6,7
                    nc.sync.dma_start(ca_out["s22"].ap()[:, 4608:EXTC],
                                      CA["s22"][:, 4608:EXTC])
            nc.sync.dma_start(ca_out["s22"].ap()[:, 0:2304],
                              CA["s22"][:, 0:2304])
            nc.sync.dma_start(ca_out["s22"].ap()[:, 2304:4096],
                              CA["s22"][:, 2304:4096])

            # ---------------- s11 (tail-critical: progressive DMA) ------
            for i, s in enumerate(ORDER["sym"][:-1]):
                run_slot("s11", 0, 0, s)
                if i == 4:
                    nc.sync.dma_start(ca_out["s11"].ap()[:, 4608:EXTC],
                                      CA["s11"][:, 4608:EXTC])
            # last slot (3): DMA each region right after it finalizes
            exp_group("s11", 0, 0, 3, 384, G, _acc_col("s11", 3, 0),
                      False)                         # [384, 2432)
            nc.sync.dma_start(ca_out["s11"].ap()[:, 0:1216],
                              CA["s11"][:, 0:1216])
            nc.sync.dma_start(ca_out["s11"].ap()[:, 1216:2432],
                              CA["s11"][:, 1216:2432])
            exp_group("s11", 0, 0, 3, 384 + G, G, _acc_col("s11", 3, 1),
                      False)                         # [2432, 4480)
            nc.sync.dma_start(ca_out["s11"].ap()[:, 2432:4096],
                              CA["s11"][:, 2432:4096])
            rag_group("s22", 1)                    # [4096, 4608)
            nc.sync.dma_start(ca_out["s22"].ap()[:, 4096:4608],
                              CA["s22"][:, 4096:4608])
            rag_group("s11", 0)                    # [4096, 4608)
            nc.scalar.dma_start(ca_out["s11"].ap()[:, 4096:4608],
                                CA["s11"][:, 4096:4608])
            nc.scalar.dma_start(acc_out.ap()[:, :], acc[:])

    nc.compile()
    return nc


_CACHE = {}


def _compiled(n_cores=N_CORES):
    if n_cores not in _CACHE:
        _CACHE[n_cores] = build_nc(n_cores)
    return _CACHE[n_cores]


def _perm(c):
    p = np.arange(EXTC)
    return TI * ((4 * c + p // TI) % T) + p % TI


def _quantize(H):
    H = np.asarray(H, np.float32)
    z = H / np.maximum(np.sqrt((H * H).sum(1, keepdims=True)), 1e-12)
    q = (z * SCALE).astype(ml_dtypes.float8_e4m3)
    return z, q


def make_in_maps(H_1, H_2, n_cores=N_CORES):
    _, q1 = _quantize(H_1)
    _, q2 = _quantize(H_2)
    # [N, D] -> [D, N] -> [2, TI, N] -> [TI, 2, N]
    qt1 = q1.T.reshape(2, TI, N_FULL).transpose(1, 0, 2)
    qt2 = q2.T.reshape(2, TI, N_FULL).transpose(1, 0, 2)
    maps = []
    for c in range(n_cores):
        pm = _perm(c)
        maps.append({"M1": np.ascontiguousarray(qt1[:, :, pm]),
                     "M2": np.ascontiguousarray(qt2[:, :, pm])})
    return maps


def finalize(results, H_1, H_2, n_cores=N_CORES):
    N = N_FULL
    z1, q1 = _quantize(H_1)
    z2, q2 = _quantize(H_2)
    den1 = np.zeros(N, np.float64)
    den2 = np.zeros(N, np.float64)
    for c in range(n_cores):
        r = results[c]
        pm = _perm(c)
        A = np.asarray(r["accs"], np.float64)
        for s in range(8):
            rt = 4 * c + s if s < 4 else 32 + 4 * c + (s - 4)
            gr = slice(TI * rt, TI * (rt + 1))
            den1[gr] += A[:, [_acc_col("s12", s, g)
                              for g in range(4)]].sum(1)
            den1[gr] += A[:, [_acc_col("s11", s, g) for g in range(2)]].sum(1)
            den2[gr] += A[:, [_acc_col("s22", s, g) for g in range(2)]].sum(1)
            if s < 4:
                den1[gr] += A[:, _rag_col("s11", s)]
                den2[gr] += A[:, _rag_col("s22", s)]
        np.add.at(den2, pm[:N],
                  np.asarray(r["ca12"], np.float64).sum(0))
        np.add.at(den1, pm, np.asarray(r["ca11"], np.float64).sum(0))
        np.add.at(den2, pm, np.asarray(r["ca22"], np.float64).sum(0))
    # exact diagonal corrections for the quantized Gram diagonals
    qf1 = q1.astype(np.float64)
    qf2 = q2.astype(np.float64)
    den1 -= np.exp(ACT_SCALE * (qf1 * qf1).sum(1))
    den2 -= np.exp(ACT_SCALE * (qf2 * qf2).sum(1))
    ii = float((z1.astype(np.float64) * z2.astype(np.float64)).sum())
    loss = (np.sum(np.log(den1)) + np.sum(np.log(den2))
            - (2.0 / TAU) * ii) / (2.0 * N)
    return np.float32(loss)


def kernel(H_1, H_2):
    nc = _compiled(N_CORES)
    in_maps = make_in_maps(H_1, H_2, N_CORES)
    res = run_bass_kernel_spmd(nc, in_maps, core_ids=list(range(N_CORES)))
    return finalize(res.results, H_1, H_2, N_CORES)
